# revision 1
# baseline (speedup 1.0000x reference)
"""Distributed RoPE multi-head attention for one TRN2 chip (8 NeuronCores).

Reference op (B=2, N=2048, C=1024, H=16, D=64, fp32):
    qkv = x @ w_qkv.T + b_qkv ; rope(q), rope(k)
    attn = softmax(q k^T / sqrt(D)) ; out = (attn v) @ w_proj.T + b_proj

Sharding: tensor-parallel over heads. Core c owns heads (2c, 2c+1) for BOTH
batch elements: it computes its slice of the QKV projection, RoPE, and full
attention for its 4 (batch, head) pairs, all in "transposed" layouts
(feature on SBUF partitions, token on the free dim) so no transposes are
needed between the matmuls. An on-chip AllToAll (2 MB/core) then reshards
the attention output from head-sharded to token-sharded, and each core runs
the output projection (full w_proj) + bias for its disjoint 512-token slice.
The host only concatenates the 8 disjoint output shards.

Matmuls run as float32r (full PE rate at free-dim >= 256, fp32 storage).
"""

import os
import sys

import numpy as np

sys.path.insert(0, "/opt/trn_rl_repo")

import ml_dtypes  # noqa: E402

BF_NP = ml_dtypes.bfloat16

import concourse.bacc as bacc  # noqa: E402
import concourse.mybir as mybir  # noqa: E402
import concourse.tile as tile  # noqa: E402

B, N, C, H, D = 2, 2048, 1024, 16, 64
T = B * N                  # 4096 flattened tokens (batch-major)
NCORES = 8
HL = H // NCORES           # 2 heads per core
CL = HL * D                # 128 local channels
TS = T // NCORES           # 512-token output slice per core
SCALE = float(D) ** -0.5
KK = C // 128              # 8 contraction tiles for the qkv matmul
KT_TILES = N // 128        # 16 key tiles per (batch, head)
QT_CH = N // 512           # 4 query chunks of 512 per batch
VS = D + 1                 # v-tile row = 64 v values + a ones column (rowsum)
VST = 80                   # per-head stride in the v tile (16B-aligned for bf16)

FP = mybir.dt.float32
FR = mybir.dt.float32r
BF = mybir.dt.bfloat16
AF = mybir.ActivationFunctionType


def _build():
    from contextlib import ExitStack

    nc = bacc.Bacc(
        "TRN2",
        target_bir_lowering=False,
        debug=False,
        enable_asserts=False,
        num_devices=NCORES,
    )

    xT = nc.dram_tensor("xT", [C, T], BF, kind="ExternalInput").ap()
    wqkvT = nc.dram_tensor("wqkvT", [C, 3 * CL], BF, kind="ExternalInput").ap()
    bqkv = nc.dram_tensor("bqkv", [128, 3], FP, kind="ExternalInput").ap()
    cos2 = nc.dram_tensor("cos2", [128, T], BF, kind="ExternalInput").ap()
    sin2 = nc.dram_tensor("sin2", [128, T], BF, kind="ExternalInput").ap()
    wpT = nc.dram_tensor("wpT", [C, C], BF, kind="ExternalInput").ap()
    bproj = nc.dram_tensor("bproj", [128, 8], FP, kind="ExternalInput").ap()
    eye = nc.dram_tensor("eye", [128, 128], FP, kind="ExternalInput").ap()
    ones = nc.dram_tensor("ones", [128, T // 128 * HL], BF, kind="ExternalInput").ap()
    outT = nc.dram_tensor("outT", [C, TS], FP, kind="ExternalOutput").ap()

    with tile.TileContext(nc) as tc, ExitStack() as outer:
        pp = outer.enter_context(tc.tile_pool(name="persist", bufs=1))
        dp = outer.enter_context(tc.tile_pool(name="dram", bufs=1, space="DRAM"))
        # shared PSUM pools: "mm" slots serve both the qkv accumulations and
        # the attention score tiles; "pc" serves the V transposes and the
        # projection accumulations.  4 + 2 + 2 = 8 banks.
        psA = outer.enter_context(tc.tile_pool(name="ps_mm", bufs=4, space="PSUM"))
        psB = outer.enter_context(tc.tile_pool(name="ps_o", bufs=2, space="PSUM"))
        psC = outer.enter_context(tc.tile_pool(name="ps_c", bufs=2, space="PSUM"))
        asb = outer.enter_context(tc.tile_pool(name="exp", bufs=6))
        nsb = outer.enter_context(tc.tile_pool(name="norm", bufs=2))

        qT = pp.tile([128, T], BF, name="qT")
        kT = pp.tile([128, T], BF, name="kT")
        vT = pp.tile([128, T], FP, name="vT")
        vsb = pp.tile([128, (T // 128) * VST * HL], BF, name="vsb")
        eye_sb = pp.tile([128, 128], FP, name="eye_sb")
        nc.sync.dma_start(eye_sb[:], eye)

        a2a_in = [
            dp.tile([NCORES, D, 512], BF, name=f"a2a_in{h}") for h in range(HL)
        ]
        a2a_out = [
            dp.tile([NCORES, D, 512], BF, name=f"a2a_out{h}") for h in range(HL)
        ]

        es = ExitStack()
        p1 = es.enter_context(tc.tile_pool(name="p1", bufs=1))
        xs = es.enter_context(tc.tile_pool(name="xs", bufs=1))

        wq = p1.tile([128, KK * 3 * CL], BF, name="wq")
        for kk in range(KK):
            nc.sync.dma_start(
                wq[:, kk * 3 * CL : (kk + 1) * 3 * CL],
                wqkvT[kk * 128 : (kk + 1) * 128, :],
            )
        bq_sb = p1.tile([128, 3], FP, name="bq_sb")
        nc.sync.dma_start(bq_sb[:], bqkv)

        dma_engines = (nc.sync, nc.gpsimd, nc.scalar)
        xfull = []
        for kk in range(KK):
            xf = xs.tile([128, T], BF, name="xf", tag=f"xf{kk}")
            xfull.append(xf)
        for half in range(2):
            hs = slice(half * (T // 2), (half + 1) * (T // 2))
            for kk in range(KK):
                dma_engines[(kk + half + 1) % 3].dma_start(
                    xfull[kk][:, hs], xT[kk * 128 : (kk + 1) * 128, hs]
                )
        cos_sb = p1.tile([128, T], BF, name="cos_sb")
        sin_sb = p1.tile([128, T], BF, name="sin_sb")
        nc.gpsimd.dma_start(cos_sb[:], cos2)
        nc.scalar.dma_start(sin_sb[:], sin2)

        qkv_dst = (qT, kT, vT)

        def emit_qkv_chunk(t):
            """QKV projection + RoPE + V retile for one 512-token chunk."""
            sl = slice(t * 512, (t + 1) * 512)
            for m in range(3):
                ps = psA.tile([128, 512], FP, name="psqkv", tag="mm")
                for kk in range(KK):
                    col = kk * 3 * CL + m * 128
                    nc.tensor.matmul(
                        ps[:],
                        lhsT=wq[:, col : col + 128],
                        rhs=xfull[kk][:, sl],
                        start=(kk == 0),
                        stop=(kk == KK - 1),
                    )
                nc.vector.tensor_scalar_add(
                    qkv_dst[m][:, sl], ps[:], bq_sb[:, m : m + 1]
                )
            # RoPE (rot halves via DMA; sign folded into sin host-side)
            for tgt in (qT, kT):
                t1 = p1.tile([128, 512], FP, name="rope1", tag="rope1", bufs=3)
                t2 = p1.tile([128, 512], BF, name="rope2", tag="rope2", bufs=3)
                nc.vector.tensor_mul(t1[:], tgt[:, sl], cos_sb[:, sl])
                for g in range(HL):
                    o = g * 64
                    nc.sync.dma_start(t2[o : o + 32, :], tgt[o + 32 : o + 64, sl])
                    nc.sync.dma_start(t2[o + 32 : o + 64, :], tgt[o : o + 32, sl])
                t3 = p1.tile([128, 512], FP, name="rope3", tag="rope3", bufs=3)
                nc.vector.tensor_mul(t3[:], t2[:], sin_sb[:, sl])
                nc.vector.tensor_add(tgt[:, sl], t1[:], t3[:])
            # V retile (token-major + ones column region)
            for tj in range(4):
                ti = t * 4 + tj
                pt = psC.tile([128, 512], FP, name="ptr", tag="pc")
                nc.tensor.transpose(
                    pt[:, 0:128], vT[:, ti * 128 : (ti + 1) * 128], eye_sb[:]
                )
                base = ti * VST * HL
                nc.vector.tensor_copy(vsb[:, base : base + D], pt[:, 0:D])
                nc.vector.tensor_copy(
                    vsb[:, base + VST : base + VST + D], pt[:, D : 2 * D]
                )

        def emit_attention(h, b):
            ho = h * D
            for qc in range(QT_CH):
                qcol = b * N + qc * 512
                po = psB.tile([VS, 512], FP, name="po", tag="po")
                exq = []
                for kt in range(KT_TILES):
                    kcol = b * N + kt * 128
                    ps = psA.tile([128, 512], FP, name="ps_s", tag="mm")
                    nc.tensor.matmul(
                        ps[:],
                        lhsT=kT[ho : ho + D, kcol : kcol + 128],
                        rhs=qT[ho : ho + D, qcol : qcol + 512],
                        start=True,
                        stop=True,
                    )
                    ex = asb.tile([128, 512], BF, name="ex", tag="ex")
                    nc.scalar.activation(ex[:], ps[:], AF.Exp, scale=SCALE)
                    exq.append((kt, ex))
                    if kt >= 2:  # scores run 2 tiles ahead of attn@v
                        _emit_attnout(po, exq.pop(0), b, h, False)
                while exq:
                    last = len(exq) == 1
                    _emit_attnout(po, exq.pop(0), b, h, last)
                poc = nsb.tile([VS, 512], FP, name="poc", tag="poc", bufs=4)
                nc.vector.tensor_copy(poc[:], po[:])
                rc = nsb.tile([1, 512], FP, name="rc", tag="rc")
                nc.vector.reciprocal(rc[:], poc[D : D + 1, :])
                bc = nsb.tile([D, 512], FP, name="bc", tag="bc")
                nc.gpsimd.partition_broadcast(bc[:], rc[:])
                an = nsb.tile([D, 512], BF, name="an", tag="an")
                nc.vector.tensor_mul(an[:], poc[0:D, :], bc[:])
                j = b * QT_CH + qc
                nc.sync.dma_start(a2a_in[h][j, :, :], an[:])

        def _emit_attnout(po, kt_ex, b, h, stop):
            pkt, pex = kt_ex
            vti = (b * N) // 128 + pkt
            vcol = vti * VST * HL + h * VST
            nc.tensor.matmul(
                po[:],
                lhsT=vsb[:, vcol : vcol + VS],
                rhs=pex[:],
                start=(pkt == 0),
                stop=stop,
            )

        def emit_a2a(h):
            nc.gpsimd.collective_compute(
                "AllToAll",
                mybir.AluOpType.bypass,
                replica_groups=[list(range(NCORES))],
                ins=[a2a_in[h].opt()],
                outs=[a2a_out[h].opt()],
            )

        # ---- interleaved schedule ----
        ones_view = vsb[:].rearrange("p (t c) -> p t c", c=VST)
        nc.sync.dma_start(
            ones_view[:, :, D : D + 1],
            ones.rearrange("p (f o) -> p f o", o=1),
        )
        for t in range(4):
            emit_qkv_chunk(t)
        emit_attention(0, 0)
        for t in range(4, 8):
            emit_qkv_chunk(t)
        emit_attention(0, 1)
        emit_a2a(0)
        es.close()  # release x / weights / rope pools

        p3 = outer.enter_context(tc.tile_pool(name="p3", bufs=1))
        p3y = outer.enter_context(tc.tile_pool(name="p3y", bufs=2))
        wp = p3.tile([128, NCORES * C], BF, name="wp")
        for j in range(NCORES):
            nc.sync.dma_start(
                wp[:, j * C : (j + 1) * C], wpT[j * 128 : (j + 1) * 128, :]
            )
        bp_sb = p3.tile([128, 8], FP, name="bp_sb")
        nc.sync.dma_start(bp_sb[:], bproj)
        ga = p3.tile([128, NCORES * 512], BF, name="ga")
        y0s = []

        def emit_proj_half(h):
            for j in range(NCORES):
                nc.sync.dma_start(
                    ga[h * D : (h + 1) * D, j * 512 : (j + 1) * 512],
                    a2a_out[h][j],
                )
            for m in range(C // 128):
                py = psC.tile([128, 512], FP, name="py", tag="pc")
                for j in range(NCORES):
                    col = j * C + m * 128
                    nc.tensor.matmul(
                        py[:],
                        lhsT=wp[h * D : (h + 1) * D, col : col + 128],
                        rhs=ga[h * D : (h + 1) * D, j * 512 : (j + 1) * 512],
                        start=(j == 0),
                        stop=(j == NCORES - 1),
                    )
                if h == 0:
                    y0 = p3y.tile([128, 512], BF, name="y0", tag=f"y0_{m}", bufs=1)
                    nc.vector.tensor_copy(y0[:], py[:])
                    y0s.append(y0)
                else:
                    ysb = p3y.tile([128, 512], FP, name="ysb", tag="ysb")
                    nc.vector.scalar_tensor_tensor(
                        ysb[:], py[:], bp_sb[:, m : m + 1], y0s[m][:],
                        op0=mybir.AluOpType.add, op1=mybir.AluOpType.add,
                    )
                    nc.sync.dma_start(outT[m * 128 : (m + 1) * 128, :], ysb[:])

        emit_attention(1, 0)
        emit_proj_half(0)
        emit_attention(1, 1)
        emit_a2a(1)
        emit_proj_half(1)

    nc.compile()
    return nc


def _prep_inputs(inputs):
    """Full inputs -> per-core in_maps (all host-side, cheap reshapes)."""
    x = np.asarray(inputs["x"], dtype=np.float32)
    cos = np.asarray(inputs["cos"], dtype=np.float32)
    sin = np.asarray(inputs["sin"], dtype=np.float32)
    w_qkv = np.asarray(inputs["w_qkv"], dtype=np.float32)
    b_qkv = np.asarray(inputs["b_qkv"], dtype=np.float32)
    w_proj = np.asarray(inputs["w_proj"], dtype=np.float32)
    b_proj = np.asarray(inputs["b_proj"], dtype=np.float32)

    xT = np.ascontiguousarray(x.reshape(T, C).T).astype(BF_NP)
    cosT = cos[0, 0].T  # [64, 2048]
    sinT = sin[0, 0].T.copy()
    sinT[: D // 2] *= -1.0  # fold rotate_half's sign into sin
    cos2 = np.ascontiguousarray(np.tile(cosT, (HL, B))).astype(BF_NP)
    sin2 = np.ascontiguousarray(np.tile(sinT, (HL, B))).astype(BF_NP)
    wpT = np.ascontiguousarray(w_proj.T).astype(BF_NP)
    bp = np.ascontiguousarray(b_proj.reshape(8, 128).T)
    eye = np.eye(128, dtype=np.float32)

    in_maps = []
    for c in range(NCORES):
        rows = np.concatenate(
            [np.arange(g * C + c * CL, g * C + (c + 1) * CL) for g in range(3)]
        )
        wq = np.ascontiguousarray(w_qkv[rows].T).astype(BF_NP)  # [1024, 384]
        bq = np.ascontiguousarray(b_qkv[rows].reshape(3, CL).T)  # [128, 3]
        in_maps.append(
            {
                "xT": xT,
                "wqkvT": wq,
                "bqkv": bq,
                "cos2": cos2,
                "sin2": sin2,
                "wpT": wpT,
                "bproj": bp,
                "eye": eye,
                "ones": np.ones((128, T // 128 * HL), dtype=BF_NP),
            }
        )
    return in_maps


_NC_CACHE = None
last_results = None


def _install_ntff_hook():
    """Best-effort: register the axon NTFF profiling hook that the boot
    skipped (the image's antenv lacks axon_hooks). Trace-mode only."""
    try:
        import types

        if "antenv.axon_hooks" not in sys.modules:
            mod = types.ModuleType("antenv.axon_hooks")
            mod._hook = None
            mod.set_axon_ntff_profile_hook = lambda h: setattr(mod, "_hook", h)
            mod.get_axon_ntff_profile_hook = lambda: mod._hook
            sys.modules["antenv.axon_hooks"] = mod
            import antenv

            antenv.axon_hooks = mod
        import antenv.axon_hooks as ah

        if ah.get_axon_ntff_profile_hook() is None:
            if "/root/.axon_site" not in sys.path:
                sys.path.insert(0, "/root/.axon_site")
            from trn_agent_boot.trn_boot import _ntff_profile_via_ctypes

            hook = _ntff_profile_via_ctypes("/opt/axon/libaxon_pjrt.so")
            if hook is not None:
                ah.set_axon_ntff_profile_hook(hook)
        # artifact upload needs a bucket this sandbox doesn't have
        import concourse.bass_utils as bu

        bu.upload_artifacts = lambda tmpdir: tmpdir
    except Exception as e:  # pragma: no cover - profiling is optional
        print(f"ntff hook install failed: {e}", file=sys.stderr)


def kernel(**inputs):
    global _NC_CACHE, last_results
    from concourse.bass_utils import run_bass_kernel_spmd

    if _NC_CACHE is None:
        _NC_CACHE = _build()
    in_maps = _prep_inputs(inputs)
    trace = os.environ.get("KBENCH_TRACE", "0") == "1"
    if trace:
        _install_ntff_hook()
    res = None
    for attempt in range(3):
        try:
            res = run_bass_kernel_spmd(
                _NC_CACHE, in_maps, core_ids=list(range(NCORES)), trace=trace
            )
            break
        except Exception:
            if attempt == 2:
                raise
            import time as _time

            _time.sleep(20)
    last_results = res
    shards = [res.results[c]["outT"].T for c in range(NCORES)]  # each [512, 1024]
    y = np.concatenate(shards, axis=0).reshape(B, N, C)
    return np.ascontiguousarray(y.astype(np.float32))



# revision 7
# speedup vs baseline: 1.1578x; 1.1578x over previous
"""Distributed RoPE multi-head attention for one TRN2 chip (8 NeuronCores).

Reference op (B=2, N=2048, C=1024, H=16, D=64, fp32):
    qkv = x @ w_qkv.T + b_qkv ; rope(q), rope(k)
    attn = softmax(q k^T / sqrt(D)) ; out = (attn v) @ w_proj.T + b_proj

Sharding: tensor-parallel over heads. Core c owns heads (2c, 2c+1) for BOTH
batch elements, computing QKV projection, RoPE and attention in transposed
layouts (feature on partitions, token on free dim). Attention output is
resharded head->token via 4 fine-grained AllToAll collectives (one per
(head, batch), launched as soon as ready so they overlap compute); each
core then projects its own 512 tokens (256 per batch) with the full w_proj.

The two heads' score matmuls contract only 64 partitions each and run
concurrently on disjoint PE row strips (auto tile_position from
base_partition 0/64); their outputs share one 2-bank PSUM tile so a single
1024-wide ACTIVATE computes both heads' exp (halving activation overhead).
A tiny warm-up AllToAll at kernel start absorbs cross-core launch skew so
the first real collective doesn't stall compute.
"""

import os
import sys
from collections import deque

import numpy as np

sys.path.insert(0, "/opt/trn_rl_repo")

import ml_dtypes  # noqa: E402

BF_NP = ml_dtypes.bfloat16

import concourse.bacc as bacc  # noqa: E402
import concourse.mybir as mybir  # noqa: E402
import concourse.tile as tile  # noqa: E402

B, N, C, H, D = 2, 2048, 1024, 16, 64
T = B * N                  # 4096 flattened tokens (batch-major)
NCORES = 8
HL = H // NCORES           # 2 heads per core
CL = HL * D                # 128 local channels
SCALE = float(D) ** -0.5
KK = C // 128              # 8 contraction tiles for the qkv matmul
VS = D + 1                 # v-tile row = 64 v values + a ones column (rowsum)
VST = 80                   # per-head stride in the v tile (16B-aligned for bf16)

FP = mybir.dt.float32
BF = mybir.dt.bfloat16
AF = mybir.ActivationFunctionType


def _build():
    from contextlib import ExitStack

    nc = bacc.Bacc(
        "TRN2",
        target_bir_lowering=False,
        debug=False,
        enable_asserts=False,
        num_devices=NCORES,
    )

    xT = nc.dram_tensor("xT", [C, T], BF, kind="ExternalInput").ap()
    wqkvT = nc.dram_tensor("wqkvT", [C, 3 * CL], BF, kind="ExternalInput").ap()
    bqkv = nc.dram_tensor("bqkv", [128, 3], FP, kind="ExternalInput").ap()
    cos2 = nc.dram_tensor("cos2", [128, T], BF, kind="ExternalInput").ap()
    sin2 = nc.dram_tensor("sin2", [128, T], BF, kind="ExternalInput").ap()
    wpT = nc.dram_tensor("wpT", [C, C], BF, kind="ExternalInput").ap()
    bproj = nc.dram_tensor("bproj", [128, 8], FP, kind="ExternalInput").ap()
    eye = nc.dram_tensor("eye", [128, 128], FP, kind="ExternalInput").ap()
    ones = nc.dram_tensor("ones", [128, T // 128 * HL], BF, kind="ExternalInput").ap()
    outT = nc.dram_tensor("outT", [C, 512], FP, kind="ExternalOutput").ap()

    with tile.TileContext(nc) as tc, ExitStack() as outer:
        pp = outer.enter_context(tc.tile_pool(name="persist", bufs=1))
        dp = outer.enter_context(tc.tile_pool(name="dram", bufs=1, space="DRAM"))
        # PSUM budget (8 banks): fused A|B score tile double-buffered (4) +
        # po A/B (2) + shared qkv/transpose/proj accumulator pool (2).
        psS = outer.enter_context(tc.tile_pool(name="ps_s", bufs=2, space="PSUM"))
        psP = outer.enter_context(tc.tile_pool(name="ps_po", bufs=1, space="PSUM"))
        psQ = outer.enter_context(tc.tile_pool(name="ps_q", bufs=2, space="PSUM"))
        asb = outer.enter_context(tc.tile_pool(name="exp", bufs=3))
        nsb = outer.enter_context(tc.tile_pool(name="norm", bufs=2))

        qT = pp.tile([128, T], BF, name="qT")
        kT = pp.tile([128, T], BF, name="kT")
        vT = pp.tile([128, T], FP, name="vT")
        vsb = pp.tile([128, (T // 128) * VST * HL], BF, name="vsb")
        eye_sb = pp.tile([128, 128], FP, name="eye_sb")

        a2a_in = {}
        a2a_out = {}
        for h in range(HL):
            for b in range(B):
                a2a_in[(h, b)] = dp.tile(
                    [NCORES, D, 256], BF, name=f"a2a_in{h}{b}"
                )
                a2a_out[(h, b)] = dp.tile(
                    [NCORES, D, 256], BF, name=f"a2a_out{h}{b}"
                )
        warm_in = dp.tile([NCORES, 1, 64], BF, name="warm_in")
        warm_out = dp.tile([NCORES, 1, 64], BF, name="warm_out")
        # skew-absorbing barrier: fires immediately at kernel start on every
        # core, while the input DMAs run, so later collectives stay fast
        nc.gpsimd.collective_compute(
            "AllToAll",
            mybir.AluOpType.bypass,
            replica_groups=[list(range(NCORES))],
            ins=[warm_in.opt()],
            outs=[warm_out.opt()],
        )

        es = ExitStack()
        p1 = es.enter_context(tc.tile_pool(name="p1", bufs=1))
        xs = es.enter_context(tc.tile_pool(name="xs", bufs=1))

        wq = p1.tile([128, KK * 3 * CL], BF, name="wq")
        bq_sb = p1.tile([128, 3], FP, name="bq_sb")
        cos_sb = p1.tile([128, T], BF, name="cos_sb")
        sin_sb = p1.tile([128, T], BF, name="sin_sb")
        nc.sync.dma_start(eye_sb[:], eye)
        nc.sync.dma_start(bq_sb[:], bqkv)
        for kk in range(KK):
            nc.sync.dma_start(
                wq[:, kk * 3 * CL : (kk + 1) * 3 * CL],
                wqkvT[kk * 128 : (kk + 1) * 128, :],
            )
        ones_view = vsb[:].rearrange("p (t c) -> p t c", c=VST)
        nc.scalar.dma_start(
            ones_view[:, :, D : D + 1],
            ones.rearrange("p (f o) -> p f o", o=1),
        )

        xfull = []
        for kk in range(KK):
            xf = xs.tile([128, T], BF, name="xf", tag=f"xf{kk}")
            xfull.append(xf)
        # chunk-major loads, x/cos/sin interleaved over the 3 DMA queues so
        # chunk 0 (plus its rope tables) lands within a few microseconds
        qs = (nc.sync, nc.gpsimd, nc.scalar)
        qi = 0
        for t in range(8):
            sl = slice(t * 512, (t + 1) * 512)
            for kk in range(KK):
                qs[qi % 3].dma_start(
                    xfull[kk][:, sl], xT[kk * 128 : (kk + 1) * 128, sl]
                )
                qi += 1
            nc.gpsimd.dma_start(cos_sb[:, sl], cos2[:, sl])
            nc.scalar.dma_start(sin_sb[:, sl], sin2[:, sl])
            qi += 2

        # preload the exp table set while the DMAs run
        dmy = p1.tile([1, 16], FP, name="dmy")
        nc.scalar.activation(dmy[:], eye_sb[0:1, 0:16], AF.Exp)

        qkv_dst = (qT, kT, vT)

        def emit_qkv_m(t, m):
            """One 512-token chunk of one of the q/k/v projections."""
            sl = slice(t * 512, (t + 1) * 512)
            ps = psQ.tile([128, 512], FP, name="psq", tag="q")
            for kk in range(KK):
                col = kk * 3 * CL + m * 128
                nc.tensor.matmul(
                    ps[:],
                    lhsT=wq[:, col : col + 128],
                    rhs=xfull[kk][:, sl],
                    start=(kk == 0),
                    stop=(kk == KK - 1),
                )
            nc.vector.tensor_scalar_add(
                qkv_dst[m][:, sl], ps[:], bq_sb[:, m : m + 1]
            )

        def emit_rope(t, tgt):
            """RoPE for one 512-token chunk (rot halves via DMA; sign folded
            into sin host-side)."""
            sl = slice(t * 512, (t + 1) * 512)
            t1 = p1.tile([128, 512], FP, name="rope1", tag="rope1", bufs=3)
            t2 = p1.tile([128, 512], BF, name="rope2", tag="rope2", bufs=3)
            nc.vector.tensor_mul(t1[:], tgt[:, sl], cos_sb[:, sl])
            for g in range(HL):
                o = g * 64
                nc.sync.dma_start(t2[o : o + 32, :], tgt[o + 32 : o + 64, sl])
                nc.sync.dma_start(t2[o + 32 : o + 64, :], tgt[o : o + 32, sl])
            t3 = p1.tile([128, 512], FP, name="rope3", tag="rope3", bufs=3)
            nc.vector.tensor_mul(t3[:], t2[:], sin_sb[:, sl])
            nc.vector.tensor_add(tgt[:, sl], t1[:], t3[:])

        def emit_vtr(t):
            """V retile for one chunk: 4 transposes into token-major vsb."""
            for tj in range(4):
                ti = t * 4 + tj
                pt = psQ.tile([128, 512], FP, name="pt", tag="q")
                nc.tensor.transpose(
                    pt[:, 0:128], vT[:, ti * 128 : (ti + 1) * 128], eye_sb[:]
                )
                base = ti * VST * HL
                nc.vector.tensor_copy(vsb[:, base : base + D], pt[:, 0:D])
                nc.vector.tensor_copy(
                    vsb[:, base + VST : base + VST + D], pt[:, D : 2 * D]
                )

        def _av(po, b, kt, ex):
            for h in range(HL):
                vti = (b * N) // 128 + kt
                vcol = vti * VST * HL + h * VST
                nc.tensor.matmul(
                    po[h][:],
                    lhsT=vsb[:, vcol : vcol + VS],
                    rhs=ex[:, h * 512 : (h + 1) * 512],
                    start=(kt == 0),
                    stop=(kt == 15),
                )

        def emit_attention_qc(b, qc, fillers):
            """Scores + exp + attn@v + norm for one 512-query chunk, both
            heads interleaved: head h's score matmul runs on PE row strip
            64h and writes bank h of a fused 2-bank PSUM tile, so one
            1024-wide ACTIVATE covers both heads. fillers: callables
            emitting independent tensor-engine work."""
            qcol = b * N + qc * 512
            po = {
                h: psP.tile([VS, 512], FP, name=f"po{h}", tag=f"po{h}")
                for h in range(HL)
            }
            pend = deque()
            fill_at = {3: 0, 7: 1, 11: 2, 15: 3}
            for kt in range(16):
                kcol = b * N + kt * 128
                ps = psS.tile([128, 1024], FP, name="ps", tag="s")
                for h in range(HL):
                    ho = h * D
                    nc.tensor.matmul(
                        ps[:, h * 512 : (h + 1) * 512],
                        lhsT=kT[ho : ho + D, kcol : kcol + 128],
                        rhs=qT[ho : ho + D, qcol : qcol + 512],
                        start=True,
                        stop=True,
                    )
                ex = asb.tile([128, 1024], BF, name="ex", tag="ex")
                nc.scalar.activation(ex[:], ps[:], AF.Exp, scale=SCALE)
                pend.append((kt, ex))
                if len(pend) > 2:
                    _av(po, b, *pend.popleft())
                if kt in fill_at and fill_at[kt] < len(fillers):
                    fillers[fill_at[kt]]()
            while pend:
                _av(po, b, *pend.popleft())
            for h in range(HL):
                rc = nsb.tile([1, 512], FP, name=f"rc{h}", tag=f"rc{h}")
                nc.vector.reciprocal(rc[:], po[h][D : D + 1, :])
                bc = nsb.tile([D, 512], FP, name=f"bc{h}", tag=f"bc{h}")
                nc.gpsimd.partition_broadcast(bc[:], rc[:])
                an = nsb.tile([D, 512], BF, name=f"an{h}", tag=f"an{h}")
                nc.vector.tensor_mul(an[:], po[h][0:D, :], bc[:])
                nc.sync.dma_start(a2a_in[(h, b)][2 * qc, :, :], an[:, 0:256])
                nc.sync.dma_start(
                    a2a_in[(h, b)][2 * qc + 1, :, :], an[:, 256:512]
                )

        def emit_a2a(h, b):
            nc.gpsimd.collective_compute(
                "AllToAll",
                mybir.AluOpType.bypass,
                replica_groups=[list(range(NCORES))],
                ins=[a2a_in[(h, b)].opt()],
                outs=[a2a_out[(h, b)].opt()],
            )

        # ---- phase A: batch-0 K (all), V (all), Q (chunk 0) ----
        for t in range(4):
            emit_qkv_m(t, 1)
            emit_rope(t, kT)
        for t in range(4):
            emit_qkv_m(t, 2)
            emit_vtr(t)
        emit_qkv_m(0, 0)
        emit_rope(0, qT)

        def f_q(t):
            def f():
                emit_qkv_m(t, 0)
                emit_rope(t, qT)
            return f

        def f_k(t):
            def f():
                emit_qkv_m(t, 1)
                emit_rope(t, kT)
            return f

        def f_v(t):
            def f():
                emit_qkv_m(t, 2)
                emit_vtr(t)
            return f

        # ---- phase B: batch-0 attention; batch-1 qkv as filler ----
        fillers_b0 = [
            [f_q(1), f_k(4), f_q(4), f_v(4)],
            [f_q(2), f_k(5), f_q(5), f_v(5)],
            [f_q(3), f_k(6), f_q(6), f_v(6)],
            [f_k(7), f_q(7), f_v(7)],
        ]
        for qc in range(4):
            emit_attention_qc(0, qc, fillers_b0[qc])
        emit_a2a(0, 0)
        emit_a2a(1, 0)
        es.close()  # release x / qkv weights / rope pools

        p3 = outer.enter_context(tc.tile_pool(name="p3", bufs=1))
        p3y = outer.enter_context(tc.tile_pool(name="p3y", bufs=2))
        wp = p3.tile([128, NCORES * C], BF, name="wp")
        for j in range(NCORES):
            (nc.scalar, nc.sync)[j % 2].dma_start(
                wp[:, j * C : (j + 1) * C], wpT[j * 128 : (j + 1) * 128, :]
            )
        bp_sb = p3.tile([128, 8], FP, name="bp_sb")
        nc.scalar.dma_start(bp_sb[:], bproj)
        ga = {}

        def emit_proj_load(b):
            g = p3.tile([128, NCORES * 256], BF, name=f"ga{b}", tag=f"ga{b}")
            for m in range(NCORES):
                nc.gpsimd.dma_start(
                    g[0:D, m * 256 : (m + 1) * 256], a2a_out[(0, b)][m]
                )
                nc.gpsimd.dma_start(
                    g[D : 2 * D, m * 256 : (m + 1) * 256], a2a_out[(1, b)][m]
                )
            ga[b] = g

        def f_proj(b, mp):
            def f():
                py = psQ.tile([128, 256], FP, name="py", tag="q")
                for m in range(NCORES):
                    col = m * C + mp * 128
                    nc.tensor.matmul(
                        py[:],
                        lhsT=wp[:, col : col + 128],
                        rhs=ga[b][:, m * 256 : (m + 1) * 256],
                        start=(m == 0),
                        stop=(m == NCORES - 1),
                    )
                ysb = p3y.tile([128, 256], FP, name="ysb", tag="ysb")
                nc.vector.tensor_scalar_add(ysb[:], py[:], bp_sb[:, mp : mp + 1])
                nc.scalar.dma_start(
                    outT[mp * 128 : (mp + 1) * 128, b * 256 : (b + 1) * 256],
                    ysb[:],
                )
            return f

        # ---- phase C: batch-1 attention; batch-0 projection only in the
        # last two query chunks (collective has ~45us to land first) ----
        emit_attention_qc(1, 0, [])
        emit_proj_load(0)
        fillers_b1 = [
            [],
            [f_proj(0, 0), f_proj(0, 1), f_proj(0, 2), f_proj(0, 3)],
            [f_proj(0, 4), f_proj(0, 5), f_proj(0, 6), f_proj(0, 7)],
        ]
        for qc in range(1, 4):
            emit_attention_qc(1, qc, fillers_b1[qc - 1])
        emit_a2a(0, 1)
        emit_a2a(1, 1)

        # ---- phase D: batch-1 projection ----
        emit_proj_load(1)
        for mp in range(NCORES):
            f_proj(1, mp)()

    nc.compile()
    return nc


def _prep_inputs(inputs):
    """Full inputs -> per-core in_maps (all host-side, cheap reshapes)."""
    x = np.asarray(inputs["x"], dtype=np.float32)
    cos = np.asarray(inputs["cos"], dtype=np.float32)
    sin = np.asarray(inputs["sin"], dtype=np.float32)
    w_qkv = np.asarray(inputs["w_qkv"], dtype=np.float32)
    b_qkv = np.asarray(inputs["b_qkv"], dtype=np.float32)
    w_proj = np.asarray(inputs["w_proj"], dtype=np.float32)
    b_proj = np.asarray(inputs["b_proj"], dtype=np.float32)

    xT = np.ascontiguousarray(x.reshape(T, C).T).astype(BF_NP)
    cosT = cos[0, 0].T  # [64, 2048]
    sinT = sin[0, 0].T.copy()
    sinT[: D // 2] *= -1.0  # fold rotate_half's sign into sin
    cos2 = np.ascontiguousarray(np.tile(cosT, (HL, B))).astype(BF_NP)
    sin2 = np.ascontiguousarray(np.tile(sinT, (HL, B))).astype(BF_NP)
    wpT = np.ascontiguousarray(w_proj.T).astype(BF_NP)
    bp = np.ascontiguousarray(b_proj.reshape(8, 128).T)
    eye = np.eye(128, dtype=np.float32)

    in_maps = []
    for c in range(NCORES):
        rows = np.concatenate(
            [np.arange(g * C + c * CL, g * C + (c + 1) * CL) for g in range(3)]
        )
        wq = np.ascontiguousarray(w_qkv[rows].T).astype(BF_NP)  # [1024, 384]
        bq = np.ascontiguousarray(b_qkv[rows].reshape(3, CL).T)  # [128, 3]
        in_maps.append(
            {
                "xT": xT,
                "wqkvT": wq,
                "bqkv": bq,
                "cos2": cos2,
                "sin2": sin2,
                "wpT": wpT,
                "bproj": bp,
                "eye": eye,
                "ones": np.ones((128, T // 128 * HL), dtype=BF_NP),
            }
        )
    return in_maps


_NC_CACHE = None
last_results = None


def _install_ntff_hook():
    """Best-effort: register the axon NTFF profiling hook that the boot
    skipped (the image's antenv lacks axon_hooks). Trace-mode only."""
    try:
        import types

        if "antenv.axon_hooks" not in sys.modules:
            mod = types.ModuleType("antenv.axon_hooks")
            mod._hook = None
            mod.set_axon_ntff_profile_hook = lambda h: setattr(mod, "_hook", h)
            mod.get_axon_ntff_profile_hook = lambda: mod._hook
            sys.modules["antenv.axon_hooks"] = mod
            import antenv

            antenv.axon_hooks = mod
        import antenv.axon_hooks as ah

        if ah.get_axon_ntff_profile_hook() is None:
            if "/root/.axon_site" not in sys.path:
                sys.path.insert(0, "/root/.axon_site")
            from trn_agent_boot.trn_boot import _ntff_profile_via_ctypes

            hook = _ntff_profile_via_ctypes("/opt/axon/libaxon_pjrt.so")
            if hook is not None:
                ah.set_axon_ntff_profile_hook(hook)
        # artifact upload needs a bucket this sandbox doesn't have
        import concourse.bass_utils as bu

        bu.upload_artifacts = lambda tmpdir: tmpdir
    except Exception as e:  # pragma: no cover - profiling is optional
        print(f"ntff hook install failed: {e}", file=sys.stderr)


def kernel(**inputs):
    global _NC_CACHE, last_results
    from concourse.bass_utils import run_bass_kernel_spmd

    if _NC_CACHE is None:
        _NC_CACHE = _build()
    in_maps = _prep_inputs(inputs)
    trace = os.environ.get("KBENCH_TRACE", "0") == "1"
    if trace:
        _install_ntff_hook()
    res = None
    for attempt in range(3):
        try:
            res = run_bass_kernel_spmd(
                _NC_CACHE, in_maps, core_ids=list(range(NCORES)), trace=trace
            )
            break
        except Exception:
            if attempt == 2:
                raise
            import time as _time

            _time.sleep(20)
    last_results = res
    # core j owns batch-0 tokens [256j, 256j+256) (cols 0:256) and the same
    # batch-1 token range (cols 256:512)
    y = np.empty((B, N, C), dtype=np.float32)
    for c in range(NCORES):
        o = res.results[c]["outT"]  # [1024, 512] fp32
        y[0, 256 * c : 256 * c + 256] = o[:, 0:256].T
        y[1, 256 * c : 256 * c + 256] = o[:, 256:512].T
    return np.ascontiguousarray(y)


# revision 18
# speedup vs baseline: 1.2323x; 1.0643x over previous
"""Distributed RoPE multi-head attention for one TRN2 chip (8 NeuronCores).

Reference op (B=2, N=2048, C=1024, H=16, D=64, fp32):
    qkv = x @ w_qkv.T + b_qkv ; rope(q), rope(k)
    attn = softmax(q k^T / sqrt(D)) ; out = (attn v) @ w_proj.T + b_proj

Sharding: tensor-parallel over heads. Core c owns heads (2c, 2c+1) for BOTH
batch elements, computing QKV projection, RoPE and attention in transposed
layouts (feature on partitions, token on free dim). Attention output is
resharded head->token via 16 fine-grained AllToAll collectives (one per
(head, batch, query-chunk), launched as soon as each query chunk is
normalized so they fully overlap compute); each core then projects its own
512 tokens (8 interleaved 64-token blocks per batch) with the full w_proj.

The two heads' score matmuls contract only 64 partitions each and run
concurrently on disjoint PE row strips (auto tile_position from
base_partition 0/64); their outputs share one 2-bank PSUM tile so a single
1024-wide ACTIVATE computes both heads' exp (halving activation overhead).
A tiny warm-up AllToAll at kernel start absorbs cross-core launch skew so
the first real collective doesn't stall compute.
"""

import os
import sys
from collections import deque

import numpy as np

sys.path.insert(0, "/opt/trn_rl_repo")

import ml_dtypes  # noqa: E402

BF_NP = ml_dtypes.bfloat16

import concourse.bacc as bacc  # noqa: E402
import concourse.mybir as mybir  # noqa: E402
import concourse.tile as tile  # noqa: E402

B, N, C, H, D = 2, 2048, 1024, 16, 64
T = B * N                  # 4096 flattened tokens (batch-major)
NCORES = 8
HL = H // NCORES           # 2 heads per core
CL = HL * D                # 128 local channels
SCALE = float(D) ** -0.5
KK = C // 128              # 8 contraction tiles for the qkv matmul
VS = D + 1                 # v-tile row = 64 v values + a ones column (rowsum)
VST = 80                   # per-head stride in the v tile (16B-aligned for bf16)

FP = mybir.dt.float32
BF = mybir.dt.bfloat16
AF = mybir.ActivationFunctionType


def _build():
    from contextlib import ExitStack

    nc = bacc.Bacc(
        "TRN2",
        target_bir_lowering=False,
        debug=False,
        enable_asserts=False,
        num_devices=NCORES,
    )

    xT = nc.dram_tensor("xT", [C, T], BF, kind="ExternalInput").ap()
    wqkvT = nc.dram_tensor("wqkvT", [C, 3 * CL], BF, kind="ExternalInput").ap()
    bqkv = nc.dram_tensor("bqkv", [128, 3], FP, kind="ExternalInput").ap()
    cos2 = nc.dram_tensor("cos2", [128, T], BF, kind="ExternalInput").ap()
    sin2 = nc.dram_tensor("sin2", [128, T], BF, kind="ExternalInput").ap()
    wpT = nc.dram_tensor("wpT", [C, C], BF, kind="ExternalInput").ap()
    bproj = nc.dram_tensor("bproj", [128, 8], FP, kind="ExternalInput").ap()
    eye = nc.dram_tensor("eye", [128, 128], FP, kind="ExternalInput").ap()
    ones = nc.dram_tensor("ones", [128, T // 128 * HL], BF, kind="ExternalInput").ap()
    outT = nc.dram_tensor("outT", [C, 512], FP, kind="ExternalOutput").ap()

    with tile.TileContext(nc) as tc, ExitStack() as outer:
        pp = outer.enter_context(tc.tile_pool(name="persist", bufs=1))
        dp = outer.enter_context(tc.tile_pool(name="dram", bufs=1, space="DRAM"))
        # PSUM budget (8 banks): fused A|B score tile double-buffered (4) +
        # po A/B (2) + shared qkv/transpose/proj accumulator pool (2).
        psS = outer.enter_context(tc.tile_pool(name="ps_s", bufs=2, space="PSUM"))
        psP = outer.enter_context(tc.tile_pool(name="ps_po", bufs=1, space="PSUM"))
        psQ = outer.enter_context(tc.tile_pool(name="ps_q", bufs=2, space="PSUM"))
        asb = outer.enter_context(tc.tile_pool(name="exp", bufs=3))
        nsb = outer.enter_context(tc.tile_pool(name="norm", bufs=2))

        qT = pp.tile([128, T], BF, name="qT")
        kT = pp.tile([128, T], BF, name="kT")
        vT = pp.tile([128, T], FP, name="vT")
        vsb = pp.tile([128, (T // 128) * VST * HL], BF, name="vsb")
        eye_sb = pp.tile([128, 128], FP, name="eye_sb")

        a2a_in = {}
        a2a_out = {}
        for h in range(HL):
            for b in range(B):
                for qc in range(4):
                    a2a_in[(h, b, qc)] = dp.tile(
                        [NCORES, D, 64], BF, name=f"a2a_in{h}{b}{qc}"
                    )
                    a2a_out[(h, b, qc)] = dp.tile(
                        [NCORES, D, 64], BF, name=f"a2a_out{h}{b}{qc}"
                    )
        warm_in = dp.tile([NCORES, 1, 64], BF, name="warm_in")
        warm_out = dp.tile([NCORES, 1, 64], BF, name="warm_out")
        # skew-absorbing barrier: fires immediately at kernel start on every
        # core, while the input DMAs run, so later collectives stay fast
        nc.gpsimd.collective_compute(
            "AllToAll",
            mybir.AluOpType.bypass,
            replica_groups=[list(range(NCORES))],
            ins=[warm_in.opt()],
            outs=[warm_out.opt()],
        )

        es = ExitStack()
        p1 = es.enter_context(tc.tile_pool(name="p1", bufs=1))
        xs = es.enter_context(tc.tile_pool(name="xs", bufs=1))

        wq = p1.tile([128, KK * 3 * CL], BF, name="wq")
        bq_sb = p1.tile([128, 3], FP, name="bq_sb")
        cos_sb = p1.tile([128, T], BF, name="cos_sb")
        sin_sb = p1.tile([128, T], BF, name="sin_sb")
        nc.sync.dma_start(eye_sb[:], eye)
        nc.sync.dma_start(bq_sb[:], bqkv)
        for kk in range(KK):
            nc.sync.dma_start(
                wq[:, kk * 3 * CL : (kk + 1) * 3 * CL],
                wqkvT[kk * 128 : (kk + 1) * 128, :],
            )
        ones_view = vsb[:].rearrange("p (t c) -> p t c", c=VST)
        nc.scalar.dma_start(
            ones_view[:, :, D : D + 1],
            ones.rearrange("p (f o) -> p f o", o=1),
        )

        xfull = []
        for kk in range(KK):
            xf = xs.tile([128, T], BF, name="xf", tag=f"xf{kk}")
            xfull.append(xf)
        # batch-half-major loads in large 512 KB DMAs (issue-rate, not
        # bandwidth, limits many small DMAs); batch-0 halves land first
        qs = (nc.sync, nc.gpsimd, nc.scalar)
        qi = 0
        for half in range(2):
            hs = slice(half * 2048, (half + 1) * 2048)
            for kk in range(KK):
                qs[qi % 3].dma_start(
                    xfull[kk][:, hs], xT[kk * 128 : (kk + 1) * 128, hs]
                )
                qi += 1
            nc.gpsimd.dma_start(cos_sb[:, hs], cos2[:, hs])
            nc.scalar.dma_start(sin_sb[:, hs], sin2[:, hs])

        # preload the exp table set while the DMAs run
        dmy = p1.tile([1, 16], FP, name="dmy")
        nc.scalar.activation(dmy[:], eye_sb[0:1, 0:16], AF.Exp)

        qkv_dst = (qT, kT, vT)

        def emit_qkv_m(t, m):
            """One 512-token chunk of one of the q/k/v projections."""
            sl = slice(t * 512, (t + 1) * 512)
            ps = psQ.tile([128, 512], FP, name="psq", tag="q")
            for kk in range(KK):
                col = kk * 3 * CL + m * 128
                nc.tensor.matmul(
                    ps[:],
                    lhsT=wq[:, col : col + 128],
                    rhs=xfull[kk][:, sl],
                    start=(kk == 0),
                    stop=(kk == KK - 1),
                )
            nc.vector.tensor_scalar_add(
                qkv_dst[m][:, sl], ps[:], bq_sb[:, m : m + 1]
            )

        def emit_rope(t, tgt):
            """RoPE for one 512-token chunk (rot halves via DMA; sign folded
            into sin host-side)."""
            sl = slice(t * 512, (t + 1) * 512)
            t1 = p1.tile([128, 512], FP, name="rope1", tag="rope1", bufs=3)
            t2 = p1.tile([128, 512], BF, name="rope2", tag="rope2", bufs=3)
            nc.vector.tensor_mul(t1[:], tgt[:, sl], cos_sb[:, sl])
            for g in range(HL):
                o = g * 64
                nc.sync.dma_start(t2[o : o + 32, :], tgt[o + 32 : o + 64, sl])
                nc.sync.dma_start(t2[o + 32 : o + 64, :], tgt[o : o + 32, sl])
            t3 = p1.tile([128, 512], FP, name="rope3", tag="rope3", bufs=3)
            nc.vector.tensor_mul(t3[:], t2[:], sin_sb[:, sl])
            nc.vector.tensor_add(tgt[:, sl], t1[:], t3[:])

        def emit_vtr(t):
            """V retile for one chunk: 4 transposes into token-major vsb."""
            for tj in range(4):
                ti = t * 4 + tj
                pt = psQ.tile([128, 512], FP, name="pt", tag="q")
                nc.tensor.transpose(
                    pt[:, 0:128], vT[:, ti * 128 : (ti + 1) * 128], eye_sb[:]
                )
                base = ti * VST * HL
                nc.vector.tensor_copy(vsb[:, base : base + D], pt[:, 0:D])
                nc.vector.tensor_copy(
                    vsb[:, base + VST : base + VST + D], pt[:, D : 2 * D]
                )

        def _av(po, b, kt, ex):
            for h in range(HL):
                vti = (b * N) // 128 + kt
                vcol = vti * VST * HL + h * VST
                nc.tensor.matmul(
                    po[h][:],
                    lhsT=vsb[:, vcol : vcol + VS],
                    rhs=ex[:, h * 512 : (h + 1) * 512],
                    start=(kt == 0),
                    stop=(kt == 15),
                )

        def emit_attention_qc(b, qc, fillers):
            """Scores + exp + attn@v + norm for one 512-query chunk, both
            heads interleaved: head h's score matmul runs on PE row strip
            64h and writes bank h of a fused 2-bank PSUM tile, so one
            1024-wide ACTIVATE covers both heads. fillers: callables
            emitting independent tensor-engine work."""
            qcol = b * N + qc * 512
            po = {
                h: psP.tile([VS, 512], FP, name=f"po{h}", tag=f"po{h}")
                for h in range(HL)
            }
            pend = deque()
            fill_at = {3: 0, 7: 1, 11: 2, 15: 3}
            for kt in range(16):
                kcol = b * N + kt * 128
                ps = psS.tile([128, 1024], FP, name="ps", tag="s")
                for h in range(HL):
                    ho = h * D
                    nc.tensor.matmul(
                        ps[:, h * 512 : (h + 1) * 512],
                        lhsT=kT[ho : ho + D, kcol : kcol + 128],
                        rhs=qT[ho : ho + D, qcol : qcol + 512],
                        start=True,
                        stop=True,
                    )
                ex = asb.tile([128, 1024], BF, name="ex", tag="ex")
                nc.scalar.activation(ex[:], ps[:], AF.Exp, scale=SCALE)
                pend.append((kt, ex))
                if len(pend) > 2:
                    _av(po, b, *pend.popleft())
                if kt in fill_at and fill_at[kt] < len(fillers):
                    fillers[fill_at[kt]]()
            while pend:
                _av(po, b, *pend.popleft())
            # softmax denominators: stage the PSUM rowsum row in SBUF, then
            # the fast 2-pass Newton reciprocal (the plain DVE reciprocal is
            # ~5x slower; approx_fast NaNs when fed PSUM directly)
            for h in range(HL):
                den = nsb.tile([1, 512], FP, name=f"den{h}", tag=f"den{h}")
                nc.vector.tensor_copy(den[:], po[h][D : D + 1, :])
                rc = nsb.tile([1, 512], FP, name=f"rc{h}", tag=f"rc{h}")
                nc.vector.reciprocal_approx_fast(rc[:], den[:])
                bc = nsb.tile([D, 512], FP, name=f"bc{h}", tag=f"bc{h}")
                nc.gpsimd.partition_broadcast(bc[:], rc[:])
                an = nsb.tile([D, 512], BF, name=f"an{h}", tag=f"an{h}")
                nc.vector.tensor_mul(an[:], po[h][0:D, :], bc[:])
                nc.sync.dma_start(
                    a2a_in[(h, b, qc)][:].rearrange("c p f -> p c f"),
                    an[:].rearrange("p (c f) -> p c f", c=NCORES),
                )
                emit_a2a(h, b, qc)

        def emit_a2a(h, b, qc):
            nc.gpsimd.collective_compute(
                "AllToAll",
                mybir.AluOpType.bypass,
                replica_groups=[list(range(NCORES))],
                ins=[a2a_in[(h, b, qc)].opt()],
                outs=[a2a_out[(h, b, qc)].opt()],
            )

        # ---- phase A: batch-0 K (all), V (all), Q (chunk 0) ----
        for t in range(4):
            emit_qkv_m(t, 1)
            emit_rope(t, kT)
        for t in range(4):
            emit_qkv_m(t, 2)
            emit_vtr(t)
        emit_qkv_m(0, 0)
        emit_rope(0, qT)

        def f_q(t):
            def f():
                emit_qkv_m(t, 0)
                emit_rope(t, qT)
            return f

        def f_k(t):
            def f():
                emit_qkv_m(t, 1)
                emit_rope(t, kT)
            return f

        def f_v(t):
            def f():
                emit_qkv_m(t, 2)
                emit_vtr(t)
            return f

        # ---- phase B: batch-0 attention; batch-1 qkv as filler ----
        fillers_b0 = [
            [f_q(1), f_k(4), f_q(4), f_v(4)],
            [f_q(2), f_k(5), f_q(5), f_v(5)],
            [f_q(3), f_k(6), f_q(6), f_v(6)],
            [f_k(7), f_q(7), f_v(7)],
        ]
        for qc in range(4):
            emit_attention_qc(0, qc, fillers_b0[qc])
        es.close()  # release x / qkv weights / rope pools

        p3 = outer.enter_context(tc.tile_pool(name="p3", bufs=1))
        p3y = outer.enter_context(tc.tile_pool(name="p3y", bufs=2))
        wp = p3.tile([128, NCORES * C], BF, name="wp")
        for j in range(NCORES):
            (nc.scalar, nc.sync)[j % 2].dma_start(
                wp[:, j * C : (j + 1) * C], wpT[j * 128 : (j + 1) * 128, :]
            )
        bp_sb = p3.tile([128, 8], FP, name="bp_sb")
        nc.scalar.dma_start(bp_sb[:], bproj)
        ga = {}

        def emit_proj_load(b):
            # ga column m*256 + qc*64 + i = head pair of source core m,
            # query chunk qc, own-token i
            g = p3.tile([128, NCORES * 256], BF, name=f"ga{b}", tag=f"ga{b}")
            for h in range(HL):
                gv = g[h * D : (h + 1) * D, :].rearrange(
                    "p (m f) -> p m f", f=256
                )
                for qc in range(4):
                    nc.gpsimd.dma_start(
                        gv[:, :, qc * 64 : (qc + 1) * 64],
                        a2a_out[(h, b, qc)][:].rearrange("m p f -> p m f"),
                    )
            ga[b] = g

        def f_proj(b, mp):
            def f():
                py = psQ.tile([128, 256], FP, name="py", tag="q")
                for m in range(NCORES):
                    col = m * C + mp * 128
                    nc.tensor.matmul(
                        py[:],
                        lhsT=wp[:, col : col + 128],
                        rhs=ga[b][:, m * 256 : (m + 1) * 256],
                        start=(m == 0),
                        stop=(m == NCORES - 1),
                    )
                ysb = p3y.tile([128, 256], FP, name="ysb", tag="ysb")
                nc.vector.tensor_scalar_add(ysb[:], py[:], bp_sb[:, mp : mp + 1])
                nc.scalar.dma_start(
                    outT[mp * 128 : (mp + 1) * 128, b * 256 : (b + 1) * 256],
                    ysb[:],
                )
            return f

        # ---- phase C: batch-1 attention; batch-0 projection as filler
        # (batch-0 collectives already completed during phase B) ----
        emit_proj_load(0)
        fillers_b1 = [
            [],
            [f_proj(0, 0), f_proj(0, 1), f_proj(0, 2)],
            [f_proj(0, 3), f_proj(0, 4), f_proj(0, 5)],
            [f_proj(0, 6), f_proj(0, 7)],
        ]
        for qc in range(4):
            emit_attention_qc(1, qc, fillers_b1[qc])

        # ---- phase D: batch-1 projection ----
        emit_proj_load(1)
        for mp in range(NCORES):
            f_proj(1, mp)()

    nc.compile()
    return nc


def _prep_inputs(inputs):
    """Full inputs -> per-core in_maps (all host-side, cheap reshapes)."""
    x = np.asarray(inputs["x"], dtype=np.float32)
    cos = np.asarray(inputs["cos"], dtype=np.float32)
    sin = np.asarray(inputs["sin"], dtype=np.float32)
    w_qkv = np.asarray(inputs["w_qkv"], dtype=np.float32)
    b_qkv = np.asarray(inputs["b_qkv"], dtype=np.float32)
    w_proj = np.asarray(inputs["w_proj"], dtype=np.float32)
    b_proj = np.asarray(inputs["b_proj"], dtype=np.float32)

    xT = np.ascontiguousarray(x.reshape(T, C).T).astype(BF_NP)
    cosT = cos[0, 0].T  # [64, 2048]
    sinT = sin[0, 0].T.copy()
    sinT[: D // 2] *= -1.0  # fold rotate_half's sign into sin
    cos2 = np.ascontiguousarray(np.tile(cosT, (HL, B))).astype(BF_NP)
    sin2 = np.ascontiguousarray(np.tile(sinT, (HL, B))).astype(BF_NP)
    wpT = np.ascontiguousarray(w_proj.T).astype(BF_NP)
    bp = np.ascontiguousarray(b_proj.reshape(8, 128).T)
    eye = np.eye(128, dtype=np.float32)

    in_maps = []
    for c in range(NCORES):
        rows = np.concatenate(
            [np.arange(g * C + c * CL, g * C + (c + 1) * CL) for g in range(3)]
        )
        wq = np.ascontiguousarray(w_qkv[rows].T).astype(BF_NP)  # [1024, 384]
        bq = np.ascontiguousarray(b_qkv[rows].reshape(3, CL).T)  # [128, 3]
        in_maps.append(
            {
                "xT": xT,
                "wqkvT": wq,
                "bqkv": bq,
                "cos2": cos2,
                "sin2": sin2,
                "wpT": wpT,
                "bproj": bp,
                "eye": eye,
                "ones": np.ones((128, T // 128 * HL), dtype=BF_NP),
            }
        )
    return in_maps


_NC_CACHE = None
last_results = None


def _install_ntff_hook():
    """Best-effort: register the axon NTFF profiling hook that the boot
    skipped (the image's antenv lacks axon_hooks). Trace-mode only."""
    try:
        import types

        if "antenv.axon_hooks" not in sys.modules:
            mod = types.ModuleType("antenv.axon_hooks")
            mod._hook = None
            mod.set_axon_ntff_profile_hook = lambda h: setattr(mod, "_hook", h)
            mod.get_axon_ntff_profile_hook = lambda: mod._hook
            sys.modules["antenv.axon_hooks"] = mod
            import antenv

            antenv.axon_hooks = mod
        import antenv.axon_hooks as ah

        if ah.get_axon_ntff_profile_hook() is None:
            if "/root/.axon_site" not in sys.path:
                sys.path.insert(0, "/root/.axon_site")
            from trn_agent_boot.trn_boot import _ntff_profile_via_ctypes

            hook = _ntff_profile_via_ctypes("/opt/axon/libaxon_pjrt.so")
            if hook is not None:
                ah.set_axon_ntff_profile_hook(hook)
        # artifact upload needs a bucket this sandbox doesn't have
        import concourse.bass_utils as bu

        bu.upload_artifacts = lambda tmpdir: tmpdir
    except Exception as e:  # pragma: no cover - profiling is optional
        print(f"ntff hook install failed: {e}", file=sys.stderr)


def kernel(**inputs):
    global _NC_CACHE, last_results
    from concourse.bass_utils import run_bass_kernel_spmd

    if _NC_CACHE is None:
        _NC_CACHE = _build()
    in_maps = _prep_inputs(inputs)
    trace = os.environ.get("KBENCH_TRACE", "0") == "1"
    if trace:
        _install_ntff_hook()
    res = None
    for attempt in range(3):
        try:
            res = run_bass_kernel_spmd(
                _NC_CACHE, in_maps, core_ids=list(range(NCORES)), trace=trace
            )
            break
        except Exception:
            if attempt == 2:
                raise
            import time as _time

            _time.sleep(20)
    last_results = res
    # core c's outT col b*256 + qc*64 + i holds batch b, token qc*512+64c+i
    y = np.empty((B, N, C), dtype=np.float32)
    for c in range(NCORES):
        o = res.results[c]["outT"]  # [1024, 512] fp32
        for b in range(B):
            blk = o[:, b * 256 : (b + 1) * 256].reshape(C, 4, 64)
            for qc in range(4):
                s = qc * 512 + 64 * c
                y[b, s : s + 64] = blk[:, qc, :].T
    return np.ascontiguousarray(y)


# revision 21
# speedup vs baseline: 1.3392x; 1.0868x over previous
"""Distributed RoPE multi-head attention for one TRN2 chip (8 NeuronCores).

Reference op (B=2, N=2048, C=1024, H=16, D=64, fp32):
    qkv = x @ w_qkv.T + b_qkv ; rope(q), rope(k)
    attn = softmax(q k^T / sqrt(D)) ; out = (attn v) @ w_proj.T + b_proj

Sharding: tensor-parallel over heads. Core c owns heads (2c, 2c+1) for BOTH
batch elements, computing QKV projection, RoPE and attention in transposed
layouts (feature on partitions, token on free dim). Attention output is
resharded head->token via 16 fine-grained AllToAll collectives (one per
(head, batch, query-chunk), launched as soon as each query chunk is
normalized so they fully overlap compute); each core then projects its own
512 tokens (8 interleaved 64-token blocks per batch) with the full w_proj.

The two heads' score matmuls contract only 64 partitions each and run
concurrently on disjoint PE row strips (auto tile_position from
base_partition 0/64); their outputs share one 2-bank PSUM tile so a single
1024-wide ACTIVATE computes both heads' exp (halving activation overhead).
A tiny warm-up AllToAll at kernel start absorbs cross-core launch skew so
the first real collective doesn't stall compute.
"""

import os
import sys
from collections import deque

import numpy as np

sys.path.insert(0, "/opt/trn_rl_repo")

import ml_dtypes  # noqa: E402

BF_NP = ml_dtypes.bfloat16

import concourse.bacc as bacc  # noqa: E402
import concourse.mybir as mybir  # noqa: E402
import concourse.tile as tile  # noqa: E402

B, N, C, H, D = 2, 2048, 1024, 16, 64
T = B * N                  # 4096 flattened tokens (batch-major)
NCORES = 8
HL = H // NCORES           # 2 heads per core
CL = HL * D                # 128 local channels
SCALE = float(D) ** -0.5
KK = C // 128              # 8 contraction tiles for the qkv matmul
VS = D + 1                 # v-tile row = 64 v values + a ones column (rowsum)
VST = 80                   # per-head stride in the v tile (16B-aligned for bf16)

FP = mybir.dt.float32
BF = mybir.dt.bfloat16
AF = mybir.ActivationFunctionType


def _build():
    from contextlib import ExitStack

    nc = bacc.Bacc(
        "TRN2",
        target_bir_lowering=False,
        debug=False,
        enable_asserts=False,
        num_devices=NCORES,
    )

    xT = nc.dram_tensor("xT", [C, T], BF, kind="ExternalInput").ap()
    wqkvT = nc.dram_tensor("wqkvT", [C, 3 * CL], BF, kind="ExternalInput").ap()
    bqkv = nc.dram_tensor("bqkv", [128, 3], FP, kind="ExternalInput").ap()
    cos2 = nc.dram_tensor("cos2", [128, T], BF, kind="ExternalInput").ap()
    sin2 = nc.dram_tensor("sin2", [128, T], BF, kind="ExternalInput").ap()
    wpT = nc.dram_tensor("wpT", [C, C], BF, kind="ExternalInput").ap()
    bproj = nc.dram_tensor("bproj", [128, 8], FP, kind="ExternalInput").ap()
    eye = nc.dram_tensor("eye", [128, 128], FP, kind="ExternalInput").ap()
    outT = nc.dram_tensor("outT", [C, 512], FP, kind="ExternalOutput").ap()

    with tile.TileContext(nc) as tc, ExitStack() as outer:
        pp = outer.enter_context(tc.tile_pool(name="persist", bufs=1))
        dp = outer.enter_context(tc.tile_pool(name="dram", bufs=1, space="DRAM"))
        # PSUM budget (8 banks): fused A|B score tile double-buffered (4) +
        # po A/B (2) + shared qkv/transpose/proj accumulator pool (2).
        psS = outer.enter_context(tc.tile_pool(name="ps_s", bufs=2, space="PSUM"))
        psP = outer.enter_context(tc.tile_pool(name="ps_po", bufs=1, space="PSUM"))
        psQ = outer.enter_context(tc.tile_pool(name="ps_q", bufs=2, space="PSUM"))
        asb = outer.enter_context(tc.tile_pool(name="exp", bufs=3))
        nsb = outer.enter_context(tc.tile_pool(name="norm", bufs=2))

        qT = pp.tile([128, T], BF, name="qT")
        kT = pp.tile([128, T], BF, name="kT")
        vT = pp.tile([128, T], FP, name="vT")
        vsb = pp.tile([128, (T // 128) * VST * HL], BF, name="vsb")
        eye_sb = pp.tile([128, 128], FP, name="eye_sb")

        a2a_in = {}
        a2a_out = {}
        for h in range(HL):
            for b in range(B):
                for qp in range(2):
                    a2a_in[(h, b, qp)] = dp.tile(
                        [NCORES, D, 128], BF, name=f"a2a_in{h}{b}{qp}"
                    )
                    a2a_out[(h, b, qp)] = dp.tile(
                        [NCORES, D, 128], BF, name=f"a2a_out{h}{b}{qp}"
                    )
        warm_in = dp.tile([NCORES, 1, 64], BF, name="warm_in")
        warm_out = dp.tile([NCORES, 1, 64], BF, name="warm_out")
        # skew-absorbing barrier: fires immediately at kernel start on every
        # core, while the input DMAs run, so later collectives stay fast
        nc.gpsimd.collective_compute(
            "AllToAll",
            mybir.AluOpType.bypass,
            replica_groups=[list(range(NCORES))],
            ins=[warm_in.opt()],
            outs=[warm_out.opt()],
        )

        es = ExitStack()
        p1 = es.enter_context(tc.tile_pool(name="p1", bufs=1))
        xs = es.enter_context(tc.tile_pool(name="xs", bufs=1))

        wq = p1.tile([128, KK * 3 * CL], BF, name="wq")
        bq_sb = p1.tile([128, 3], FP, name="bq_sb")
        cos_sb = p1.tile([128, T], BF, name="cos_sb")
        sin_sb = p1.tile([128, T], BF, name="sin_sb")
        nc.sync.dma_start(eye_sb[:], eye)
        nc.sync.dma_start(bq_sb[:], bqkv)
        for kk in range(KK):
            nc.sync.dma_start(
                wq[:, kk * 3 * CL : (kk + 1) * 3 * CL],
                wqkvT[kk * 128 : (kk + 1) * 128, :],
            )
        ones_view = vsb[:].rearrange("p (t c) -> p t c", c=VST)
        nc.gpsimd.memset(ones_view[:, :, D : D + 1], 1.0)

        xfull = []
        for kk in range(KK):
            xf = xs.tile([128, T], BF, name="xf", tag=f"xf{kk}")
            xfull.append(xf)
        # batch-half-major loads in large 512 KB DMAs (issue-rate, not
        # bandwidth, limits many small DMAs); batch-0 halves land first
        qs = (nc.sync, nc.scalar)
        qi = 0
        for half in range(2):
            hs = slice(half * 2048, (half + 1) * 2048)
            for kk in range(KK):
                qs[qi % 2].dma_start(
                    xfull[kk][:, hs], xT[kk * 128 : (kk + 1) * 128, hs]
                )
                qi += 1
            nc.gpsimd.dma_start(cos_sb[:, hs], cos2[:, hs])
            nc.gpsimd.dma_start(sin_sb[:, hs], sin2[:, hs])

        # preload the exp table set while the DMAs run
        dmy = p1.tile([1, 16], FP, name="dmy")
        nc.scalar.activation(dmy[:], eye_sb[0:1, 0:16], AF.Exp)

        qkv_dst = (qT, kT, vT)

        def emit_qkv_m(t, m):
            """One 512-token chunk of one of the q/k/v projections."""
            sl = slice(t * 512, (t + 1) * 512)
            ps = psQ.tile([128, 512], FP, name="psq", tag="q")
            for kk in range(KK):
                col = kk * 3 * CL + m * 128
                nc.tensor.matmul(
                    ps[:],
                    lhsT=wq[:, col : col + 128],
                    rhs=xfull[kk][:, sl],
                    start=(kk == 0),
                    stop=(kk == KK - 1),
                )
            nc.vector.tensor_scalar_add(
                qkv_dst[m][:, sl], ps[:], bq_sb[:, m : m + 1]
            )

        def emit_rope(t, tgt):
            """RoPE for one 512-token chunk (rot halves via DMA; sign folded
            into sin host-side)."""
            sl = slice(t * 512, (t + 1) * 512)
            t1 = p1.tile([128, 512], FP, name="rope1", tag="rope1", bufs=3)
            t2 = p1.tile([128, 512], BF, name="rope2", tag="rope2", bufs=3)
            nc.vector.tensor_mul(t1[:], tgt[:, sl], cos_sb[:, sl])
            for g in range(HL):
                o = g * 64
                nc.vector.tensor_copy(t2[o : o + 32, :], tgt[o + 32 : o + 64, sl])
                nc.vector.tensor_copy(t2[o + 32 : o + 64, :], tgt[o : o + 32, sl])
            t3 = p1.tile([128, 512], FP, name="rope3", tag="rope3", bufs=3)
            nc.vector.tensor_mul(t3[:], t2[:], sin_sb[:, sl])
            nc.vector.tensor_add(tgt[:, sl], t1[:], t3[:])

        def emit_vtr(t):
            """V retile for one chunk: 4 transposes into token-major vsb."""
            for tj in range(4):
                ti = t * 4 + tj
                pt = psQ.tile([128, 512], FP, name="pt", tag="q")
                nc.tensor.transpose(
                    pt[:, 0:128], vT[:, ti * 128 : (ti + 1) * 128], eye_sb[:]
                )
                base = ti * VST * HL
                nc.vector.tensor_copy(vsb[:, base : base + D], pt[:, 0:D])
                nc.vector.tensor_copy(
                    vsb[:, base + VST : base + VST + D], pt[:, D : 2 * D]
                )

        def _av(po, b, kt, ex):
            for h in range(HL):
                vti = (b * N) // 128 + kt
                vcol = vti * VST * HL + h * VST
                nc.tensor.matmul(
                    po[h][:],
                    lhsT=vsb[:, vcol : vcol + VS],
                    rhs=ex[:, h * 512 : (h + 1) * 512],
                    start=(kt == 0),
                    stop=(kt == 15),
                )

        def emit_attention_qc(b, qc, fillers):
            """Scores + exp + attn@v + norm for one 512-query chunk, both
            heads interleaved: head h's score matmul runs on PE row strip
            64h and writes bank h of a fused 2-bank PSUM tile, so one
            1024-wide ACTIVATE covers both heads. fillers: callables
            emitting independent tensor-engine work."""
            qcol = b * N + qc * 512
            po = {
                h: psP.tile([VS, 512], FP, name=f"po{h}", tag=f"po{h}")
                for h in range(HL)
            }
            pend = deque()
            fill_at = {3: 0, 7: 1, 11: 2, 15: 3}
            for kt in range(16):
                kcol = b * N + kt * 128
                ps = psS.tile([128, 1024], FP, name="ps", tag="s")
                for h in range(HL):
                    ho = h * D
                    nc.tensor.matmul(
                        ps[:, h * 512 : (h + 1) * 512],
                        lhsT=kT[ho : ho + D, kcol : kcol + 128],
                        rhs=qT[ho : ho + D, qcol : qcol + 512],
                        start=True,
                        stop=True,
                    )
                ex = asb.tile([128, 1024], BF, name="ex", tag="ex")
                nc.scalar.activation(ex[:], ps[:], AF.Exp, scale=SCALE)
                pend.append((kt, ex))
                if len(pend) > 2:
                    _av(po, b, *pend.popleft())
                if kt in fill_at and fill_at[kt] < len(fillers):
                    fillers[fill_at[kt]]()
            while pend:
                _av(po, b, *pend.popleft())
            # softmax denominators: stage the PSUM rowsum row in SBUF, then
            # the fast 2-pass Newton reciprocal (the plain DVE reciprocal is
            # ~5x slower; approx_fast NaNs when fed PSUM directly)
            for h in range(HL):
                den = nsb.tile([1, 512], FP, name=f"den{h}", tag=f"den{h}")
                nc.vector.tensor_copy(den[:], po[h][D : D + 1, :])
                rc = nsb.tile([1, 512], FP, name=f"rc{h}", tag=f"rc{h}")
                nc.vector.reciprocal_approx_fast(rc[:], den[:])
                bc = nsb.tile([D, 512], FP, name=f"bc{h}", tag=f"bc{h}")
                nc.gpsimd.partition_broadcast(bc[:], rc[:])
                an = nsb.tile([D, 512], BF, name=f"an{h}", tag=f"an{h}")
                nc.vector.tensor_mul(an[:], po[h][0:D, :], bc[:])
                qp, hf = qc // 2, qc % 2
                nc.sync.dma_start(
                    a2a_in[(h, b, qp)][hf * 4 : hf * 4 + 4].rearrange(
                        "c p f -> p c f"
                    ),
                    an[:].rearrange("p (c f) -> p c f", c=4),
                )
                if hf == 1:
                    emit_a2a(h, b, qp)

        def emit_a2a(h, b, qc):
            nc.gpsimd.collective_compute(
                "AllToAll",
                mybir.AluOpType.bypass,
                replica_groups=[list(range(NCORES))],
                ins=[a2a_in[(h, b, qc)].opt()],
                outs=[a2a_out[(h, b, qc)].opt()],
            )

        # ---- phase A: batch-0 K (all), V (all), Q (chunk 0) ----
        for t in range(4):
            emit_qkv_m(t, 1)
            emit_rope(t, kT)
        for t in range(4):
            emit_qkv_m(t, 2)
            emit_vtr(t)
        emit_qkv_m(0, 0)
        emit_rope(0, qT)

        def f_q(t):
            def f():
                emit_qkv_m(t, 0)
                emit_rope(t, qT)
            return f

        def f_k(t):
            def f():
                emit_qkv_m(t, 1)
                emit_rope(t, kT)
            return f

        def f_v(t):
            def f():
                emit_qkv_m(t, 2)
                emit_vtr(t)
            return f

        # ---- phase B: batch-0 attention; batch-1 qkv as filler ----
        fillers_b0 = [
            [f_q(1), f_k(4), f_q(4), f_v(4)],
            [f_q(2), f_k(5), f_q(5), f_v(5)],
            [f_q(3), f_k(6), f_q(6), f_v(6)],
            [f_k(7), f_q(7), f_v(7)],
        ]
        for qc in range(4):
            emit_attention_qc(0, qc, fillers_b0[qc])
        es.close()  # release x / qkv weights / rope pools

        p3 = outer.enter_context(tc.tile_pool(name="p3", bufs=1))
        p3y = outer.enter_context(tc.tile_pool(name="p3y", bufs=2))
        wp = p3.tile([128, NCORES * C], BF, name="wp")
        for j in range(NCORES):
            (nc.scalar, nc.sync)[j % 2].dma_start(
                wp[:, j * C : (j + 1) * C], wpT[j * 128 : (j + 1) * 128, :]
            )
        bp_sb = p3.tile([128, 8], FP, name="bp_sb")
        nc.scalar.dma_start(bp_sb[:], bproj)
        ga = {}

        def emit_proj_load(b, qps):
            # ga column m*256 + qp*128 + i = head pair of source core m,
            # query-chunk pair qp, own-token i
            if b not in ga:
                ga[b] = p3.tile(
                    [128, NCORES * 256], BF, name=f"ga{b}", tag=f"ga{b}"
                )
            g = ga[b]
            for h in range(HL):
                gv = g[h * D : (h + 1) * D, :].rearrange(
                    "p (m f) -> p m f", f=256
                )
                for qp in qps:
                    nc.gpsimd.dma_start(
                        gv[:, :, qp * 128 : (qp + 1) * 128],
                        a2a_out[(h, b, qp)][:].rearrange("m p f -> p m f"),
                    )

        def f_proj(b, mp, qps):
            def f():
                w = 128 * len(qps)
                c0 = qps[0] * 128
                py = psQ.tile([128, w], FP, name="py", tag="q", padded_shape=[128, 512])
                for m in range(NCORES):
                    col = m * C + mp * 128
                    nc.tensor.matmul(
                        py[:],
                        lhsT=wp[:, col : col + 128],
                        rhs=ga[b][:, m * 256 + c0 : m * 256 + c0 + w],
                        start=(m == 0),
                        stop=(m == NCORES - 1),
                    )
                ysb = p3y.tile([128, w], FP, name="ysb", tag="ysb", padded_shape=[128, 256])
                nc.vector.tensor_scalar_add(ysb[:], py[:], bp_sb[:, mp : mp + 1])
                nc.scalar.dma_start(
                    outT[mp * 128 : (mp + 1) * 128, b * 256 + c0 : b * 256 + c0 + w],
                    ysb[:],
                )
            return f

        # ---- phase C: batch-1 attention; batch-0 projection as filler
        # (batch-0 qc-pair collectives completed during phase B) ----
        emit_proj_load(0, (0, 1))
        fillers_b1 = [
            [f_proj(0, 0, (0, 1)), f_proj(0, 1, (0, 1)), f_proj(0, 2, (0, 1))],
            [f_proj(0, 3, (0, 1)), f_proj(0, 4, (0, 1)), f_proj(0, 5, (0, 1))],
            [f_proj(0, 6, (0, 1)), f_proj(0, 7, (0, 1))],
            [f_proj(1, 0, (0,)), f_proj(1, 1, (0,)), f_proj(1, 2, (0,)),
             f_proj(1, 3, (0,))],
        ]
        emit_attention_qc(1, 0, fillers_b1[0])
        emit_attention_qc(1, 1, fillers_b1[1])
        emit_proj_load(1, (0,))
        emit_attention_qc(1, 2, fillers_b1[2])
        emit_attention_qc(1, 3, fillers_b1[3])

        # ---- phase D: batch-1 qc-pair-1 projection ----
        emit_proj_load(1, (1,))
        for mp in range(4, NCORES):
            f_proj(1, mp, (0,))()
        for mp in range(NCORES):
            f_proj(1, mp, (1,))()

    nc.compile()
    return nc


def _prep_inputs(inputs):
    """Full inputs -> per-core in_maps (all host-side, cheap reshapes)."""
    x = np.asarray(inputs["x"], dtype=np.float32)
    cos = np.asarray(inputs["cos"], dtype=np.float32)
    sin = np.asarray(inputs["sin"], dtype=np.float32)
    w_qkv = np.asarray(inputs["w_qkv"], dtype=np.float32)
    b_qkv = np.asarray(inputs["b_qkv"], dtype=np.float32)
    w_proj = np.asarray(inputs["w_proj"], dtype=np.float32)
    b_proj = np.asarray(inputs["b_proj"], dtype=np.float32)

    xT = np.ascontiguousarray(x.reshape(T, C).T).astype(BF_NP)
    cosT = cos[0, 0].T  # [64, 2048]
    sinT = sin[0, 0].T.copy()
    sinT[: D // 2] *= -1.0  # fold rotate_half's sign into sin
    cos2 = np.ascontiguousarray(np.tile(cosT, (HL, B))).astype(BF_NP)
    sin2 = np.ascontiguousarray(np.tile(sinT, (HL, B))).astype(BF_NP)
    wpT = np.ascontiguousarray(w_proj.T).astype(BF_NP)
    bp = np.ascontiguousarray(b_proj.reshape(8, 128).T)
    eye = np.eye(128, dtype=np.float32)

    in_maps = []
    for c in range(NCORES):
        rows = np.concatenate(
            [np.arange(g * C + c * CL, g * C + (c + 1) * CL) for g in range(3)]
        )
        wq = np.ascontiguousarray(w_qkv[rows].T).astype(BF_NP)  # [1024, 384]
        bq = np.ascontiguousarray(b_qkv[rows].reshape(3, CL).T)  # [128, 3]
        in_maps.append(
            {
                "xT": xT,
                "wqkvT": wq,
                "bqkv": bq,
                "cos2": cos2,
                "sin2": sin2,
                "wpT": wpT,
                "bproj": bp,
                "eye": eye,
            }
        )
    return in_maps


_NC_CACHE = None
last_results = None


def _install_ntff_hook():
    """Best-effort: register the axon NTFF profiling hook that the boot
    skipped (the image's antenv lacks axon_hooks). Trace-mode only."""
    try:
        import types

        if "antenv.axon_hooks" not in sys.modules:
            mod = types.ModuleType("antenv.axon_hooks")
            mod._hook = None
            mod.set_axon_ntff_profile_hook = lambda h: setattr(mod, "_hook", h)
            mod.get_axon_ntff_profile_hook = lambda: mod._hook
            sys.modules["antenv.axon_hooks"] = mod
            import antenv

            antenv.axon_hooks = mod
        import antenv.axon_hooks as ah

        if ah.get_axon_ntff_profile_hook() is None:
            if "/root/.axon_site" not in sys.path:
                sys.path.insert(0, "/root/.axon_site")
            from trn_agent_boot.trn_boot import _ntff_profile_via_ctypes

            hook = _ntff_profile_via_ctypes("/opt/axon/libaxon_pjrt.so")
            if hook is not None:
                ah.set_axon_ntff_profile_hook(hook)
        # artifact upload needs a bucket this sandbox doesn't have
        import concourse.bass_utils as bu

        bu.upload_artifacts = lambda tmpdir: tmpdir
    except Exception as e:  # pragma: no cover - profiling is optional
        print(f"ntff hook install failed: {e}", file=sys.stderr)


def kernel(**inputs):
    global _NC_CACHE, last_results
    from concourse.bass_utils import run_bass_kernel_spmd

    if _NC_CACHE is None:
        _NC_CACHE = _build()
    in_maps = _prep_inputs(inputs)
    trace = os.environ.get("KBENCH_TRACE", "0") == "1"
    if trace:
        _install_ntff_hook()
    res = None
    for attempt in range(3):
        try:
            res = run_bass_kernel_spmd(
                _NC_CACHE, in_maps, core_ids=list(range(NCORES)), trace=trace
            )
            break
        except Exception:
            if attempt == 2:
                raise
            import time as _time

            _time.sleep(20)
    last_results = res
    # core c's outT col b*256 + qp*128 + i holds batch b, token qp*1024+128c+i
    y = np.empty((B, N, C), dtype=np.float32)
    for c in range(NCORES):
        o = res.results[c]["outT"]  # [1024, 512] fp32
        for b in range(B):
            for qp in range(2):
                s = qp * 1024 + 128 * c
                y[b, s : s + 128] = o[:, b * 256 + qp * 128 : b * 256 + (qp + 1) * 128].T
    return np.ascontiguousarray(y)
